# revision 6
# baseline (speedup 1.0000x reference)
"""KoLeo loss kernel for Trainium2, 8-core SPMD.

Math: for L2-normalized x, ||x_i - x_j||^2 = 2 - 2<x_i,x_j>, so the loss
  -mean(log(||x_i - x_nn(i)|| + eps))
needs only the per-row MAX inner product (diagonal masked), not the argmax:
  loss = -mean(log(sqrt(2 - 2*m_i) + eps)),  m_i = max_{j!=i} <x_i, x_j>

Sharding: row-parallel over the 16384x16384 similarity matrix; core c owns
rows [c*2048, (c+1)*2048). Each core receives the full x^T (fp16, [256,16384])
ROTATED by -c*2048 columns so that its own row block is always at columns
0:2048 — this makes the diagonal-mask positions identical on every core
(same NEFF on all cores). Each core emits a partial sum of logs; the host
combines: loss = -(sum of partials)/N.

Per-core device pipeline:
  - dots tile (r,j) = xT[:, r*128:+128].T @ xT[:, j*512:+512], K=256 as two
    128-chunk fp16 matmuls accumulated in one PSUM bank (fp32).
  - diagonal fix: one extra matmul accumulating -2*I at the block's own
    columns (dots[i,i] = 1 - 2 = -1, matching the reference's mask).
  - PSUM drain split across engines (GPSIMD cannot read PSUM):
      * ~44% of tiles: DVE reduce_max PSUM -> [128,1] slot
      * rest: ACT copies PSUM -> fp16 SBUF scratch, GPSIMD running
        elementwise max into a per-row-tile [128,512] fp16 accumulator
  - finale: combine maxes, d = sqrt(2-2m) (ACT), ln(d+1e-8) (ACT),
    row reduce_sum (DVE), partition sum via ones-matmul (PE), DMA out.

fp16 input quantization gives dot error ~6e-5 (fp32 PSUM accumulation is
exact); resulting loss relative error ~3e-4.
"""
import sys

if "/opt/trn_rl_repo" not in sys.path:
    sys.path.insert(0, "/opt/trn_rl_repo")

from contextlib import ExitStack

import numpy as np

import concourse.bass as bass  # noqa: F401  (AP types used implicitly)
import concourse.mybir as mybir
import concourse.tile as tile
from concourse import bacc, bass_utils

N_CORES = 8
N = 16384            # total rows
D = 256              # feature dim
BLK = N // N_CORES   # 2048 rows per core
R = BLK // 128       # 16 row tiles per core
CT = 512             # column tile width
J = N // CT          # 32 column tiles
EPS = 1e-8
F16 = mybir.dt.float16
F32 = mybir.dt.float32
# Tile drain split: DVE reduce_max straight from PSUM has no DVE fast mode
# (658ns/tile); tensor_tensor(max) on all-fp16 SBUF operands runs in 2x mode
# (~330ns/tile); ACT activation-copy PSUM->fp16 SBUF costs ~600ns/tile.
# GPSIMD ucode has no max op and no PSUM port, so it only does memsets.
# 5/16 direct DVE + 11/16 (ACT copy + DVE fp16 max) balances DVE ~220us,
# ACT ~212us, both under the PE matmul span.
DVE_MOD = 16
DVE_CNT = 5

_STATE: dict = {}


def _build_nc(j_run=J):
    nc = bacc.Bacc("TRN2", target_bir_lowering=False, debug=False,
                   num_devices=N_CORES)
    xt = nc.dram_tensor("xt", [D, N], F16, kind="ExternalInput").ap()
    ident = nc.dram_tensor("ident", [128, 128], F16, kind="ExternalInput").ap()
    diag4 = nc.dram_tensor("diag4", [128, 4 * CT], F16, kind="ExternalInput").ap()
    ones = nc.dram_tensor("ones", [128, 1], F32, kind="ExternalInput").ap()
    out = nc.dram_tensor("partial", [1, 1], F32, kind="ExternalOutput").ap()

    with tile.TileContext(nc) as tc, ExitStack() as ctx:
        cst = ctx.enter_context(tc.tile_pool(name="cst", bufs=1))
        xtp = ctx.enter_context(tc.tile_pool(name="xtp", bufs=1))
        runp = ctx.enter_context(tc.tile_pool(name="runp", bufs=1))
        scr = ctx.enter_context(tc.tile_pool(name="scr", bufs=4))
        fin = ctx.enter_context(tc.tile_pool(name="fin", bufs=1))
        psd = ctx.enter_context(tc.tile_pool(name="psd", bufs=6, space="PSUM"))
        psf = ctx.enter_context(tc.tile_pool(name="psf", bufs=1, space="PSUM"))

        ident_sb = cst.tile([128, 128], F16, tag="ident")
        nc.sync.dma_start(ident_sb[:], ident[:])
        diag4_sb = cst.tile([128, 4 * CT], F16, tag="diag4")
        nc.sync.dma_start(diag4_sb[:], diag4[:])
        ones_sb = cst.tile([128, 1], F32, tag="ones")
        nc.sync.dma_start(ones_sb[:], ones[:])

        # x^T in SBUF as per-column-tile chunks (lo = features 0:128,
        # hi = 128:256) so matmuls can start before the full 8MB lands.
        xlo = [xtp.tile([128, CT], F16, tag=f"lo{j}", name=f"lo{j}") for j in range(J)]
        xhi = [xtp.tile([128, CT], F16, tag=f"hi{j}", name=f"hi{j}") for j in range(J)]
        # weight region (the core's own block = chunks 0..3) first
        order = list(range(4)) + list(range(4, J))
        for j in order:
            nc.sync.dma_start(xlo[j][:], xt[0:128, j * CT:(j + 1) * CT])
            nc.sync.dma_start(xhi[j][:], xt[128:256, j * CT:(j + 1) * CT])

        # [128, R*J] slots for DVE-direct per-tile maxes (unwritten slots
        # stay at -4, below any real dot in [-1,1])
        maxcols = fin.tile([128, R * J], F32, tag="maxcols")
        nc.gpsimd.memset(maxcols[:], -4.0)
        rung = [runp.tile([128, CT], F16, tag=f"rg{r}", name=f"rg{r}") for r in range(R)]
        for r in range(R):
            nc.gpsimd.memset(rung[r][:], -4.0)

        for j in range(j_run):
            for r in range(R):
                ps = psd.tile([128, CT], F32, tag="dots")
                lo_w = xlo[r // 4][:, (r % 4) * 128:(r % 4 + 1) * 128]
                hi_w = xhi[r // 4][:, (r % 4) * 128:(r % 4 + 1) * 128]
                diag = (j == r // 4)
                nc.tensor.matmul(ps[:], lo_w, xlo[j][:], start=True, stop=False)
                nc.tensor.matmul(ps[:], hi_w, xhi[j][:], start=False,
                                 stop=not diag)
                if diag:
                    v = r % 4
                    nc.tensor.matmul(ps[:], ident_sb[:],
                                     diag4_sb[:, v * CT:(v + 1) * CT],
                                     start=False, stop=True)
                if (j + r) % DVE_MOD < DVE_CNT:
                    slot = r * J + j
                    nc.vector.reduce_max(maxcols[:, slot:slot + 1], ps[:],
                                         axis=mybir.AxisListType.X)
                else:
                    sc = scr.tile([128, CT], F16, tag="scr")
                    nc.scalar.copy(sc[:], ps[:])
                    nc.vector.tensor_max(rung[r][:], rung[r][:], sc[:])

        # finale
        b2 = cst.tile([128, 1], F32, tag="b2")
        nc.gpsimd.memset(b2[:], 2.0)
        beps = cst.tile([128, 1], F32, tag="beps")
        nc.gpsimd.memset(beps[:], EPS)
        maxes = fin.tile([128, R], F32, tag="maxes")
        for r in range(R):
            ta = scr.tile([128, 1], F32, tag="ta")
            tb = scr.tile([128, 1], F32, tag="tb")
            nc.vector.reduce_max(ta[:], maxcols[:, r * J:(r + 1) * J],
                                 axis=mybir.AxisListType.X)
            nc.vector.reduce_max(tb[:], rung[r][:], axis=mybir.AxisListType.X)
            nc.vector.tensor_max(maxes[:, r:r + 1], ta[:], tb[:])
        dist = fin.tile([128, R], F32, tag="dist")
        nc.scalar.activation(dist[:], maxes[:],
                             mybir.ActivationFunctionType.Sqrt,
                             bias=b2[:], scale=-2.0)
        lg = fin.tile([128, R], F32, tag="lg")
        nc.scalar.activation(lg[:], dist[:],
                             mybir.ActivationFunctionType.Ln,
                             bias=beps[:], scale=1.0)
        s = fin.tile([128, 1], F32, tag="s")
        nc.vector.reduce_sum(s[:], lg[:], axis=mybir.AxisListType.X)
        pf = psf.tile([1, 1], F32, tag="pf")
        nc.tensor.matmul(pf[:], ones_sb[:], s[:], start=True, stop=True)
        res = fin.tile([1, 1], F32, tag="res")
        nc.vector.tensor_copy(res[:], pf[:])
        nc.sync.dma_start(out[:], res[:])

    nc.compile()
    return nc


def _get_nc():
    if "nc" not in _STATE:
        _STATE["nc"] = _build_nc()
    return _STATE["nc"]


def _prepare(x: np.ndarray) -> list[dict]:
    """Host-side sharding: rotated fp16 x^T per core + constants."""
    x = np.asarray(x, dtype=np.float32)
    assert x.shape == (N, D)
    xt16 = np.ascontiguousarray(x.T).astype(np.float16)  # [256, 16384]
    ident = np.eye(128, dtype=np.float16)
    diag4 = np.zeros((128, 4 * CT), dtype=np.float16)
    for v in range(4):
        idx = np.arange(128)
        diag4[idx, v * CT + v * 128 + idx] = -2.0
    ones = np.ones((128, 1), dtype=np.float32)
    in_maps = []
    for c in range(N_CORES):
        s = c * BLK
        if s == 0:
            xr = xt16
        else:
            xr = np.concatenate([xt16[:, s:], xt16[:, :s]], axis=1)
        in_maps.append({
            "xt": np.ascontiguousarray(xr),
            "ident": ident,
            "diag4": diag4,
            "ones": ones,
        })
    return in_maps


def _combine(results: list[dict]) -> np.ndarray:
    total = sum(float(results[c]["partial"][0, 0]) for c in range(N_CORES))
    return np.asarray(-total / N, dtype=np.float32)


def kernel(normalized_feature: np.ndarray) -> np.ndarray:
    nc = _get_nc()
    in_maps = _prepare(normalized_feature)
    res = bass_utils.run_bass_kernel_spmd(nc, in_maps,
                                          core_ids=list(range(N_CORES)))
    return _combine(res.results)
